# revision 1
# baseline (speedup 1.0000x reference)
"""Grouped SwiGLU MoE MLP (16 experts) on 8 NeuronCores, expert-parallel.

Reference computation, per expert e over its contiguous token slice xi:
    out = (silu(xi @ w_gate[e].T) * (xi @ w_up[e].T)) @ w_down[e].T

Sharding: expert-parallel. Core c owns experts {2c, 2c+1}; the host hands it
the matching contiguous 2048-token slice of x (tokens are pre-sorted by
expert), so no device-side collectives are needed. Everything is handed to
the device feature-major (transposed on host) so the token axis is the
matmul moving/free dimension:

  gateT[f,t] = sum_h wgT[h,f] * xT[h,t]      (PE: lhsT=wgT tile, rhs=xT)
  hidT[f,t]  = silu(gateT) * upT             (ACT silu + DVE mul)
  outT[h,t]  = sum_f wdT[f,h] * hidT[f,t]    (PE: lhsT=wdT tile, rhs=hidT)

float32r matmuls (full PE rate at N>=512 free dim), fp32 PSUM accumulation.
Weights stream through SBUF in >=1 MiB chunks; x and hidden stay resident.
"""

import numpy as np

import concourse.bass as bass
import concourse.bacc as bacc
import concourse.mybir as mybir
from concourse import tile
from concourse.bass_utils import run_bass_kernel_spmd

E, T, H, F = 16, 16384, 1024, 2048
NCORES = 8
EPC = E // NCORES          # experts per core
TPE = T // E               # tokens per expert (uniform fast path)
P = 128                    # SBUF partitions
HT = H // P                # 8 h-tiles (contraction tiles for gate/up)
FT = F // P                # 16 f-tiles
HGS = H // P               # 8 output h-groups for down proj
NT = 512                   # matmul moving free dim (PSUM bank = 512 fp32)
TH = TPE // NT             # 2 t-halves
FG = 8                     # f-groups for gate/up weight streaming
FPG = FT // FG             # f-tiles per group = 2
FGW = F // FG              # f columns per group = 256

_F32 = mybir.dt.float32
_F32R = mybir.dt.float32r

_CACHE = {}

# Set by run for test harness introspection (exec_time_ns, profile).
LAST_RESULTS = None
TRACE = False
TRACE_KW = {}
# "silu" uses the native ScalarE Silu LUT; "sigmoid" decomposes it as
# gate*sigmoid(gate) for CoreSim, which lacks a Silu implementation.
ACT_MODE = "silu"


def _build_nc():
    nc = bacc.Bacc()
    xt_d = nc.dram_tensor("xt", [EPC, H, TPE], _F32R, kind="ExternalInput")
    wg_d = nc.dram_tensor("wg", [EPC, H, F], _F32R, kind="ExternalInput")
    wu_d = nc.dram_tensor("wu", [EPC, H, F], _F32R, kind="ExternalInput")
    wd_d = nc.dram_tensor("wd", [EPC, F, H], _F32R, kind="ExternalInput")
    out_d = nc.dram_tensor("outT", [EPC, H, TPE], _F32, kind="ExternalOutput")

    with tile.TileContext(nc) as tc:
        with (
            tc.tile_pool(name="xp", bufs=8) as xp,
            tc.tile_pool(name="wgp", bufs=3) as wgp,
            tc.tile_pool(name="wup", bufs=3) as wup,
            tc.tile_pool(name="wdp", bufs=3) as wdp,
            tc.tile_pool(name="hid", bufs=FT + 1) as hidp,
            tc.tile_pool(name="tmp", bufs=3) as tmpp,
            tc.tile_pool(name="osb", bufs=3) as osbp,
            tc.tile_pool(name="ps", bufs=8, space=bass.MemorySpace.PSUM) as psp,
        ):
            for el in range(EPC):
                # DRAM views with the h-tile index split out of the partition
                # axis: [128p, HT, F].
                wg_v = wg_d[el].rearrange("(a p) f -> p a f", p=P)
                wu_v = wu_d[el].rearrange("(a p) f -> p a f", p=P)

                # fg0's weight chunks go out ahead of the bulk x load so the
                # first matmul chain can start as soon as x's first h-tile
                # lands instead of behind the whole 4 MiB of x.
                fsl0 = slice(0, FGW)
                wgt0 = wgp.tile([P, HT, FGW], _F32R, tag="wg")
                nc.sync.dma_start(wgt0[:], wg_v[:, :, fsl0])
                wut0 = wup.tile([P, HT, FGW], _F32R, tag="wu")
                nc.sync.dma_start(wut0[:], wu_v[:, :, fsl0])

                # Token activations, resident for the whole expert: 8 tiles
                # [128h, 1024t], alternated across the two HWDGE rings so the
                # startup fill isn't serialized on one ring's FIFO.
                xts = []
                for ht in range(HT):
                    xt = xp.tile([P, TPE], _F32R, tag="xt")
                    dma_eng = nc.sync if ht % 2 == 0 else nc.scalar
                    dma_eng.dma_start(xt[:], xt_d[el, ht * P:(ht + 1) * P, :])
                    xts.append(xt)

                hidden = [hidp.tile([P, TPE], _F32R, tag="hid", name=f"hid{el}_{i}") for i in range(FT)]

                for fgi in range(FG):
                    if fgi == 0:
                        wgt, wut = wgt0, wut0
                    else:
                        fsl = slice(fgi * FGW, (fgi + 1) * FGW)
                        wgt = wgp.tile([P, HT, FGW], _F32R, tag="wg")
                        nc.sync.dma_start(wgt[:], wg_v[:, :, fsl])
                        wut = wup.tile([P, HT, FGW], _F32R, tag="wu")
                        nc.sync.dma_start(wut[:], wu_v[:, :, fsl])

                    gate_ps, up_ps = {}, {}
                    for wt, store in ((wgt, gate_ps), (wut, up_ps)):
                        for ftl in range(FPG):
                            for th in range(TH):
                                store[ftl, th] = psp.tile([P, NT], _F32, tag="ps", name="gu_ps")
                            for ht in range(HT):
                                lhsT = wt[:, ht, ftl * P:(ftl + 1) * P]
                                for th in range(TH):
                                    nc.tensor.matmul(
                                        store[ftl, th][:],
                                        lhsT,
                                        xts[ht][:, th * NT:(th + 1) * NT],
                                        start=(ht == 0),
                                        stop=(ht == HT - 1),
                                    )
                    for ftl in range(FPG):
                        ft = fgi * FPG + ftl
                        for th in range(TH):
                            tsl = slice(th * NT, (th + 1) * NT)
                            tmp = tmpp.tile([P, NT], _F32, tag="tmp")
                            if ACT_MODE == "silu":
                                nc.scalar.activation(
                                    tmp[:], gate_ps[ftl, th][:],
                                    mybir.ActivationFunctionType.Silu,
                                )
                            else:
                                nc.scalar.activation(
                                    tmp[:], gate_ps[ftl, th][:],
                                    mybir.ActivationFunctionType.Sigmoid,
                                )
                                nc.vector.tensor_mul(
                                    tmp[:], tmp[:], gate_ps[ftl, th][:]
                                )
                            nc.vector.tensor_mul(
                                hidden[ft][:, tsl], tmp[:], up_ps[ftl, th][:]
                            )

                # Down projection: outT[h,t] accumulating over all 16 f-tiles.
                wd_v = wd_d[el].rearrange("(a p) h -> p a h", p=P)
                for hg in range(HGS):
                    hsl = slice(hg * P, (hg + 1) * P)
                    wdt = wdp.tile([P, FT, P], _F32R, tag="wd")
                    nc.sync.dma_start(wdt[:], wd_v[:, :, hsl])
                    ops = [psp.tile([P, NT], _F32, tag="ps", name="dn_ps") for _ in range(TH)]
                    for ft in range(FT):
                        lhsT = wdt[:, ft, :]
                        for th in range(TH):
                            nc.tensor.matmul(
                                ops[th][:],
                                lhsT,
                                hidden[ft][:, th * NT:(th + 1) * NT],
                                start=(ft == 0),
                                stop=(ft == FT - 1),
                            )
                    osb = osbp.tile([P, TPE], _F32, tag="osb")
                    for th in range(TH):
                        nc.vector.tensor_copy(osb[:, th * NT:(th + 1) * NT], ops[th][:])
                    # Stores go out on the ACT HWDGE ring so they never queue
                    # behind pending weight loads on the SP ring.
                    nc.scalar.dma_start(out_d[el, hsl, :], osb[:])
    return nc


def get_nc():
    if "nc" not in _CACHE:
        nc = _build_nc()
        nc.finalize()
        _CACHE["nc"] = nc
    return _CACHE["nc"]


def make_in_maps(x, w_gate, w_up, w_down):
    in_maps = []
    for c in range(NCORES):
        e0 = c * EPC
        xs = x[e0 * TPE:(e0 + EPC) * TPE].reshape(EPC, TPE, H)
        in_maps.append({
            "xt": np.ascontiguousarray(xs.transpose(0, 2, 1)),
            "wg": np.ascontiguousarray(w_gate[e0:e0 + EPC].transpose(0, 2, 1)),
            "wu": np.ascontiguousarray(w_up[e0:e0 + EPC].transpose(0, 2, 1)),
            "wd": np.ascontiguousarray(w_down[e0:e0 + EPC].transpose(0, 2, 1)),
        })
    return in_maps


def _numpy_fallback(x, w_gate, w_up, w_down, counts):
    out = np.empty((x.shape[0], w_down.shape[1]), np.float32)
    o = 0
    for e in range(len(counts)):
        n = int(counts[e])
        xi = x[o:o + n]
        gate = xi @ w_gate[e].T
        up = xi @ w_up[e].T
        hidden = (gate / (1.0 + np.exp(-gate))) * up
        out[o:o + n] = hidden @ w_down[e].T
        o += n
    return out


def kernel(x, w_gate, w_up, w_down, tokens_per_expert):
    global LAST_RESULTS
    x = np.asarray(x, dtype=np.float32)
    w_gate = np.asarray(w_gate, dtype=np.float32)
    w_up = np.asarray(w_up, dtype=np.float32)
    w_down = np.asarray(w_down, dtype=np.float32)
    counts = np.asarray(tokens_per_expert).astype(np.int64)

    if not (counts.shape == (E,) and np.all(counts == TPE)):
        # Non-uniform routing: the compiled program is shaped for the
        # uniform split the reference generator produces.
        return _numpy_fallback(x, w_gate, w_up, w_down, counts)

    nc = get_nc()
    res = run_bass_kernel_spmd(
        nc, make_in_maps(x, w_gate, w_up, w_down), list(range(NCORES)),
        trace=TRACE, **TRACE_KW,
    )
    LAST_RESULTS = res
    out = np.empty((T, H), np.float32)
    for c in range(NCORES):
        o = res.results[c]["outT"]  # [EPC, H, TPE]
        for el in range(EPC):
            t0 = (c * EPC + el) * TPE
            out[t0:t0 + TPE] = o[el].T
    return out



# revision 2
# speedup vs baseline: 1.0646x; 1.0646x over previous
"""Grouped SwiGLU MoE MLP (16 experts) on 8 NeuronCores, expert-parallel.

Reference computation, per expert e over its contiguous token slice xi:
    out = (silu(xi @ w_gate[e].T) * (xi @ w_up[e].T)) @ w_down[e].T

Sharding: expert-parallel. Core c owns experts {2c, 2c+1}; the host hands it
the matching contiguous 2048-token slice of x (tokens are pre-sorted by
expert), so no device-side collectives are needed.

v2: all matmul operands are bf16 (host-cast; PE streaming rate is identical
to fp32r but FWL halves LDWEIGHTS time and all input DMA bytes halve).
PSUM accumulation stays fp32 and the output is fp32, so the only precision
loss is bf16 quantization of x/w/hidden (~5e-3 rel err, well inside 2e-2).

Layout: everything is pre-packed on the host so every DMA is a contiguous
block with >=1KiB per-partition rows:
  xq  [EPC, P, HT, TPE]   xq[e,p,a,t]  = x[e*TPE+t, a*P+p]
  wgq [EPC, FT, P, HT*P]  wgq[e,f,p,a*P+j] = w_gate[e, f*P+j, a*P+p]
  wuq  same as wgq for w_up
  wdq [EPC, HG, P, FT*P]  wdq[e,g,p,f*P+j] = w_down[e, g*P+j, f*P+p]
  outq[EPC, HG, P, TPE]   outq[e,g,p,t] = out[e*TPE+t, g*P+p]

Schedule notes (from the v1 trace): the framework preamble ends ~6.5us and
the first DMA packet lands ~8us, so the PE is pre-warmed with NWARM dummy
matmuls on a zeroed tile to get HAM to K=8/8 before real data arrives; the
first f-tile's weights and the th=0 half of x go out first, split across
both HWDGE rings, so real matmul chains start as soon as ~1.5MB has landed
instead of 2.5MB serialized on one ring. Output stores alternate rings so
the final two stores drain in parallel.
"""

import numpy as np
import ml_dtypes

import concourse.bass as bass
import concourse.bacc as bacc
import concourse.mybir as mybir
from concourse import tile
from concourse.bass_utils import run_bass_kernel_spmd

E, T, H, F = 16, 16384, 1024, 2048
NCORES = 8
EPC = E // NCORES          # experts per core
TPE = T // E               # tokens per expert (uniform fast path)
P = 128                    # SBUF partitions
HT = H // P                # 8 h-tiles (contraction tiles for gate/up)
FT = F // P                # 16 f-tiles
HGS = H // P               # 8 output h-groups for down proj
NT = 512                   # matmul moving free dim (PSUM bank = 512 fp32)
TH = TPE // NT             # 2 t-halves
NWARM = 16                 # dummy matmuls to pre-warm the PE clock

BF16 = mybir.dt.bfloat16
F32 = mybir.dt.float32
BF16_NP = ml_dtypes.bfloat16

_CACHE = {}

# Set by run for test harness introspection (exec_time_ns, profile).
LAST_RESULTS = None
TRACE = False
TRACE_KW = {}


def _build_nc():
    nc = bacc.Bacc()
    xq = nc.dram_tensor("xq", [EPC, P, HT, TPE], BF16, kind="ExternalInput")
    wgq = nc.dram_tensor("wgq", [EPC, FT, P, HT * P], BF16, kind="ExternalInput")
    wuq = nc.dram_tensor("wuq", [EPC, FT, P, HT * P], BF16, kind="ExternalInput")
    wdq = nc.dram_tensor("wdq", [EPC, HGS, P, FT * P], BF16, kind="ExternalInput")
    outq = nc.dram_tensor("outq", [EPC, HGS, P, TPE], F32, kind="ExternalOutput")

    with tile.TileContext(nc) as tc:
        with (
            tc.tile_pool(name="xp", bufs=2) as xp,
            tc.tile_pool(name="wgp", bufs=5) as wgp,
            tc.tile_pool(name="wup", bufs=5) as wup,
            tc.tile_pool(name="wdp", bufs=8) as wdp,
            tc.tile_pool(name="hid", bufs=FT + 1) as hidp,
            tc.tile_pool(name="tmp", bufs=5) as tmpp,
            tc.tile_pool(name="osb", bufs=4) as osbp,
            tc.tile_pool(name="ps", bufs=8, space=bass.MemorySpace.PSUM) as psp,
        ):
            # PE warm-up: the framework preamble + first DMA take ~9us, long
            # enough for HAM to hold the PE at K=4/8. Dummy matmuls on a
            # zeroed tile keep the PE busy through that window so real work
            # starts at 2.4GHz.
            warm = tmpp.tile([P, NT], BF16, tag="tmp", name="warm")
            nc.gpsimd.memset(warm[:], 0.0)
            wps = psp.tile([P, NT], F32, tag="ps", name="warm_ps")
            for _ in range(NWARM):
                nc.tensor.matmul(wps[:], warm[:, 0:P], warm[:],
                                 start=True, stop=True)

            xts = {}
            wgts = {}
            wuts = {}

            def prefetch_head(el):
                """First f-tile weights + x, split across both rings."""
                xt = xp.tile([P, HT, TPE], BF16, tag="xt", name=f"x{el}")
                xts[el] = xt
                wgt = wgp.tile([P, HT * P], BF16, tag="wg", name=f"wg{el}_0")
                nc.sync.dma_start(wgt[:], wgq[el, 0])
                wgts[(el, 0)] = wgt
                wut = wup.tile([P, HT * P], BF16, tag="wu", name=f"wu{el}_0")
                nc.scalar.dma_start(wut[:], wuq[el, 0])
                wuts[(el, 0)] = wut
                nc.sync.dma_start(xt[:, 0:4, 0:NT], xq[el][:, 0:4, 0:NT])
                nc.scalar.dma_start(xt[:, 4:8, 0:NT], xq[el][:, 4:8, 0:NT])
                nc.sync.dma_start(xt[:, 0:4, NT:TPE], xq[el][:, 0:4, NT:TPE])
                nc.scalar.dma_start(xt[:, 4:8, NT:TPE], xq[el][:, 4:8, NT:TPE])

            prefetch_head(0)
            for el in range(EPC):
                xt = xts[el]
                for ft in range(1, FT):
                    wgt = wgp.tile([P, HT * P], BF16, tag="wg", name=f"wg{el}_{ft}")
                    nc.sync.dma_start(wgt[:], wgq[el, ft])
                    wgts[(el, ft)] = wgt
                    wut = wup.tile([P, HT * P], BF16, tag="wu", name=f"wu{el}_{ft}")
                    nc.scalar.dma_start(wut[:], wuq[el, ft])
                    wuts[(el, ft)] = wut
                wdts = {}
                for hg in range(HGS):
                    wdt = wdp.tile([P, FT * P], BF16, tag="wd", name=f"wd{el}_{hg}")
                    nc.sync.dma_start(wdt[:], wdq[el, hg])
                    wdts[hg] = wdt

                # Gate/up: per f-tile, 4 PSUM chains (gate/up x th0/th1)
                # accumulate over the 8 h-tiles; silu+mul drain them into
                # the bf16 hidden tile while the next f-tile's chains run.
                hids = []
                for ft in range(FT):
                    g_ps = [psp.tile([P, NT], F32, tag="ps", name=f"g{el}_{ft}_{th}")
                            for th in range(TH)]
                    u_ps = [psp.tile([P, NT], F32, tag="ps", name=f"u{el}_{ft}_{th}")
                            for th in range(TH)]
                    wgt, wut = wgts.pop((el, ft)), wuts.pop((el, ft))
                    for ht in range(HT):
                        lg = wgt[:, ht * P:(ht + 1) * P]
                        for th in range(TH):
                            nc.tensor.matmul(
                                g_ps[th][:], lg,
                                xt[:, ht, th * NT:(th + 1) * NT],
                                start=(ht == 0), stop=(ht == HT - 1),
                            )
                        lu = wut[:, ht * P:(ht + 1) * P]
                        for th in range(TH):
                            nc.tensor.matmul(
                                u_ps[th][:], lu,
                                xt[:, ht, th * NT:(th + 1) * NT],
                                start=(ht == 0), stop=(ht == HT - 1),
                            )
                    hid = hidp.tile([P, TPE], BF16, tag="hid", name=f"hid{el}_{ft}")
                    for th in range(TH):
                        tmp = tmpp.tile([P, NT], BF16, tag="tmp")
                        nc.scalar.activation(
                            tmp[:], g_ps[th][:],
                            mybir.ActivationFunctionType.Silu,
                        )
                        nc.vector.tensor_mul(
                            hid[:, th * NT:(th + 1) * NT], tmp[:], u_ps[th][:]
                        )
                    hids.append(hid)

                # Next expert's head prefetch goes out before this expert's
                # output stores so its x/weights are resident at the
                # expert boundary.
                if el + 1 < EPC:
                    prefetch_head(el + 1)

                # Down projection: outT[h,t] accumulating over all 16
                # f-tiles. Stores alternate rings and go out per t-half so
                # the tail after the last matmul is minimal.
                for hg in range(HGS):
                    d_ps = [psp.tile([P, NT], F32, tag="ps", name=f"d{el}_{hg}_{th}")
                            for th in range(TH)]
                    wdt = wdts[hg]
                    for ft in range(FT):
                        ld = wdt[:, ft * P:(ft + 1) * P]
                        for th in range(TH):
                            nc.tensor.matmul(
                                d_ps[th][:], ld,
                                hids[ft][:, th * NT:(th + 1) * NT],
                                start=(ft == 0), stop=(ft == FT - 1),
                            )
                    for th in range(TH):
                        osb = osbp.tile([P, NT], F32, tag="osb")
                        nc.vector.tensor_copy(osb[:], d_ps[th][:])
                        eng = nc.sync if (hg * TH + th) % 2 == 0 else nc.scalar
                        eng.dma_start(
                            outq[el, hg][:, th * NT:(th + 1) * NT], osb[:]
                        )
    return nc


def get_nc():
    if "nc" not in _CACHE:
        nc = _build_nc()
        nc.finalize()
        _CACHE["nc"] = nc
    return _CACHE["nc"]


def make_in_maps(x, w_gate, w_up, w_down):
    xb = x.astype(BF16_NP)
    wgb = w_gate.astype(BF16_NP)
    wub = w_up.astype(BF16_NP)
    wdb = w_down.astype(BF16_NP)
    in_maps = []
    for c in range(NCORES):
        e0 = c * EPC
        # xq[e,p,a,t] = x[e*TPE+t, a*P+p]
        xs = xb[e0 * TPE:(e0 + EPC) * TPE].reshape(EPC, TPE, HT, P)
        xqc = np.ascontiguousarray(xs.transpose(0, 3, 2, 1))
        # wgq[e,f,p,a*P+j] = w_gate[e, f*P+j, a*P+p]
        wg = wgb[e0:e0 + EPC].reshape(EPC, FT, P, HT, P)
        wgc = np.ascontiguousarray(wg.transpose(0, 1, 4, 3, 2)).reshape(
            EPC, FT, P, HT * P)
        wu = wub[e0:e0 + EPC].reshape(EPC, FT, P, HT, P)
        wuc = np.ascontiguousarray(wu.transpose(0, 1, 4, 3, 2)).reshape(
            EPC, FT, P, HT * P)
        # wdq[e,g,p,f*P+j] = w_down[e, g*P+j, f*P+p]
        wd = wdb[e0:e0 + EPC].reshape(EPC, HGS, P, FT, P)
        wdc = np.ascontiguousarray(wd.transpose(0, 1, 4, 3, 2)).reshape(
            EPC, HGS, P, FT * P)
        in_maps.append({"xq": xqc, "wgq": wgc, "wuq": wuc, "wdq": wdc})
    return in_maps


def _numpy_fallback(x, w_gate, w_up, w_down, counts):
    out = np.empty((x.shape[0], w_down.shape[1]), np.float32)
    o = 0
    for e in range(len(counts)):
        n = int(counts[e])
        xi = x[o:o + n]
        gate = xi @ w_gate[e].T
        up = xi @ w_up[e].T
        hidden = (gate / (1.0 + np.exp(-gate))) * up
        out[o:o + n] = hidden @ w_down[e].T
        o += n
    return out


def kernel(x, w_gate, w_up, w_down, tokens_per_expert):
    global LAST_RESULTS
    x = np.asarray(x, dtype=np.float32)
    w_gate = np.asarray(w_gate, dtype=np.float32)
    w_up = np.asarray(w_up, dtype=np.float32)
    w_down = np.asarray(w_down, dtype=np.float32)
    counts = np.asarray(tokens_per_expert).astype(np.int64)

    if not (counts.shape == (E,) and np.all(counts == TPE)):
        # Non-uniform routing: the compiled program is shaped for the
        # uniform split the reference generator produces.
        return _numpy_fallback(x, w_gate, w_up, w_down, counts)

    nc = get_nc()
    res = run_bass_kernel_spmd(
        nc, make_in_maps(x, w_gate, w_up, w_down), list(range(NCORES)),
        trace=TRACE, **TRACE_KW,
    )
    LAST_RESULTS = res
    out = np.empty((T, H), np.float32)
    for c in range(NCORES):
        o = res.results[c]["outq"]  # [EPC, HGS, P, TPE]
        for el in range(EPC):
            t0 = (c * EPC + el) * TPE
            # out[t0+t, g*P+p] = o[el, g, p, t]
            out[t0:t0 + TPE] = o[el].transpose(2, 0, 1).reshape(TPE, H)
    return out


# revision 3
# speedup vs baseline: 1.0733x; 1.0082x over previous
"""Grouped SwiGLU MoE MLP (16 experts) on 8 NeuronCores, expert-parallel.

Reference computation, per expert e over its contiguous token slice xi:
    out = (silu(xi @ w_gate[e].T) * (xi @ w_up[e].T)) @ w_down[e].T

Sharding: expert-parallel. Core c owns experts {2c, 2c+1}; the host hands it
the matching contiguous 2048-token slice of x (tokens are pre-sorted by
expert), so no device-side collectives are needed.

All matmul operands are bf16 (host-cast): the PE streaming rate matches
fp32r but FWL halves LDWEIGHTS time and input DMA bytes halve. PSUM
accumulation stays fp32 and the output is fp32 (~4e-3 rel err, inside the
2e-2 gate).

Layout: pre-packed on the host so every DMA is a contiguous block:
  xq  [EPC, P, HT, TPE]   xq[e,p,a,t]  = x[e*TPE+t, a*P+p]
  wgq [EPC, FT, P, HT*P]  wgq[e,f,p,a*P+j] = w_gate[e, f*P+j, a*P+p]
  wuq  same as wgq for w_up
  wdq [EPC, HG, P, FT*P]  wdq[e,g,p,f*P+j] = w_down[e, g*P+j, f*P+p]
  outq[EPC, HG, P, TPE]   outq[e,g,p,t] = out[e*TPE+t, g*P+p]

Schedule (from trace analysis of earlier revisions):
- The framework preamble ends ~6.5us and the first DMA packet lands ~8us;
  NWARM dummy matmuls on a zeroed tile hold HAM at K=8/8 through that
  window so real work starts at 2.4GHz.
- Chains are t-half-outer so the first f-tile only needs the th=0 half of
  x (1MB instead of 2MB) before the PE can run 32 back-to-back matmuls,
  and so each t-half's PSUM pair drains while the other half's chains run.
- The Scalar queue runs ONLY the silu ACTIVATEs: a dma_start whose
  pool-pacing semaphore isn't yet satisfied parks its whole queue, and
  parking the silu queue delays PSUM consumption and costs the PE one
  matmul slot per group. All steady-state DMA goes on the Sync ring,
  ordered so no paced load ever sits ahead of a store it would block.
- Expert 0's first-tile weights + x are split across both rings for
  startup bandwidth (the Scalar queue is empty until the first silu).
"""

import numpy as np
import ml_dtypes

import concourse.bass as bass
import concourse.bacc as bacc
import concourse.mybir as mybir
from concourse import tile
from concourse.bass_utils import run_bass_kernel_spmd

E, T, H, F = 16, 16384, 1024, 2048
NCORES = 8
EPC = E // NCORES          # experts per core
TPE = T // E               # tokens per expert (uniform fast path)
P = 128                    # SBUF partitions
HT = H // P                # 8 h-tiles (contraction tiles for gate/up)
FT = F // P                # 16 f-tiles
HGS = H // P               # 8 output h-groups for down proj
NT = 512                   # matmul moving free dim (PSUM bank = 512 fp32)
TH = TPE // NT             # 2 t-halves
NWARM = 16                 # dummy matmuls to pre-warm the PE clock

BF16 = mybir.dt.bfloat16
F32 = mybir.dt.float32
BF16_NP = ml_dtypes.bfloat16

_CACHE = {}

# Set by run for test harness introspection (exec_time_ns, profile).
LAST_RESULTS = None
TRACE = False
TRACE_KW = {}


def _build_nc():
    nc = bacc.Bacc()
    xq = nc.dram_tensor("xq", [EPC, P, HT, TPE], BF16, kind="ExternalInput")
    wgq = nc.dram_tensor("wgq", [EPC, FT, P, HT * P], BF16, kind="ExternalInput")
    wuq = nc.dram_tensor("wuq", [EPC, FT, P, HT * P], BF16, kind="ExternalInput")
    wdq = nc.dram_tensor("wdq", [EPC, HGS, P, FT * P], BF16, kind="ExternalInput")
    outq = nc.dram_tensor("outq", [EPC, HGS, P, TPE], F32, kind="ExternalOutput")

    with tile.TileContext(nc) as tc:
        with (
            tc.tile_pool(name="xp", bufs=2) as xp,
            tc.tile_pool(name="wgp", bufs=5) as wgp,
            tc.tile_pool(name="wup", bufs=5) as wup,
            tc.tile_pool(name="wdp", bufs=8) as wdp,
            tc.tile_pool(name="hid", bufs=FT + 1) as hidp,
            tc.tile_pool(name="tmp", bufs=5) as tmpp,
            tc.tile_pool(name="osb", bufs=4) as osbp,
            tc.tile_pool(name="ps", bufs=8, space=bass.MemorySpace.PSUM) as psp,
        ):
            # PE warm-up (see module docstring).
            warm = tmpp.tile([P, NT], BF16, tag="tmp", name="warm")
            nc.gpsimd.memset(warm[:], 0.0)
            wps = psp.tile([P, NT], F32, tag="ps", name="warm_ps")
            for _ in range(NWARM):
                nc.tensor.matmul(wps[:], warm[:, 0:P], warm[:],
                                 start=True, stop=True)

            xts = {}
            wgts = {}
            wuts = {}

            def prefetch_head(el):
                """First f-tile weights + x. For expert 0 this is the
                startup-critical burst and is split across both rings; the
                Scalar queue is otherwise empty until the first silu."""
                xt = xp.tile([P, HT, TPE], BF16, tag="xt", name=f"x{el}")
                xts[el] = xt
                wgt = wgp.tile([P, HT * P], BF16, tag="wg", name=f"wg{el}_0")
                nc.sync.dma_start(wgt[:], wgq[el, 0])
                wgts[(el, 0)] = wgt
                wut = wup.tile([P, HT * P], BF16, tag="wu", name=f"wu{el}_0")
                nc.scalar.dma_start(wut[:], wuq[el, 0])
                wuts[(el, 0)] = wut
                nc.sync.dma_start(xt[:, 0:4, 0:NT], xq[el][:, 0:4, 0:NT])
                nc.scalar.dma_start(xt[:, 4:8, 0:NT], xq[el][:, 4:8, 0:NT])
                nc.sync.dma_start(xt[:, 0:4, NT:TPE], xq[el][:, 0:4, NT:TPE])
                nc.scalar.dma_start(xt[:, 4:8, NT:TPE], xq[el][:, 4:8, NT:TPE])

            prefetch_head(0)
            for el in range(EPC):
                xt = xts[el]
                for ft in range(1, FT):
                    wgt = wgp.tile([P, HT * P], BF16, tag="wg", name=f"wg{el}_{ft}")
                    nc.sync.dma_start(wgt[:], wgq[el, ft])
                    wgts[(el, ft)] = wgt
                    wut = wup.tile([P, HT * P], BF16, tag="wu", name=f"wu{el}_{ft}")
                    nc.sync.dma_start(wut[:], wuq[el, ft])
                    wuts[(el, ft)] = wut
                wdts = {}
                for hg in range(HGS):
                    wdt = wdp.tile([P, FT * P], BF16, tag="wd", name=f"wd{el}_{hg}")
                    nc.sync.dma_start(wdt[:], wdq[el, hg])
                    wdts[hg] = wdt

                # Gate/up, t-half-outer: for each f-tile, run the gate and
                # up chains for th=0 (16 MMs), whose silu+mul drain while
                # the th=1 chains (16 MMs) run.
                hids = []
                for ft in range(FT):
                    wgt, wut = wgts.pop((el, ft)), wuts.pop((el, ft))
                    hid = hidp.tile([P, TPE], BF16, tag="hid", name=f"hid{el}_{ft}")
                    for th in range(TH):
                        tsl = slice(th * NT, (th + 1) * NT)
                        g_ps = psp.tile([P, NT], F32, tag="ps", name=f"g{el}_{ft}_{th}")
                        u_ps = psp.tile([P, NT], F32, tag="ps", name=f"u{el}_{ft}_{th}")
                        for ht in range(HT):
                            nc.tensor.matmul(
                                g_ps[:], wgt[:, ht * P:(ht + 1) * P],
                                xt[:, ht, tsl],
                                start=(ht == 0), stop=(ht == HT - 1),
                            )
                        for ht in range(HT):
                            nc.tensor.matmul(
                                u_ps[:], wut[:, ht * P:(ht + 1) * P],
                                xt[:, ht, tsl],
                                start=(ht == 0), stop=(ht == HT - 1),
                            )
                        tmp = tmpp.tile([P, NT], BF16, tag="tmp")
                        nc.scalar.activation(
                            tmp[:], g_ps[:],
                            mybir.ActivationFunctionType.Silu,
                        )
                        nc.vector.tensor_mul(hid[:, tsl], tmp[:], u_ps[:])
                    hids.append(hid)

                # Next expert's head prefetch goes out before this expert's
                # output stores so its x/weights are resident at the
                # expert boundary.
                if el + 1 < EPC:
                    prefetch_head(el + 1)

                # Down projection, t-half-outer: th=0's 16-MM chain
                # completes before th=1's starts, so its copy+store overlap
                # the th=1 chain and the post-loop tail is one store deep.
                for hg in range(HGS):
                    wdt = wdts[hg]
                    for th in range(TH):
                        tsl = slice(th * NT, (th + 1) * NT)
                        d_ps = psp.tile([P, NT], F32, tag="ps", name=f"d{el}_{hg}_{th}")
                        for ft in range(FT):
                            nc.tensor.matmul(
                                d_ps[:], wdt[:, ft * P:(ft + 1) * P],
                                hids[ft][:, tsl],
                                start=(ft == 0), stop=(ft == FT - 1),
                            )
                        osb = osbp.tile([P, NT], F32, tag="osb")
                        nc.vector.tensor_copy(osb[:], d_ps[:])
                        nc.sync.dma_start(outq[el, hg][:, tsl], osb[:])
    return nc


def get_nc():
    if "nc" not in _CACHE:
        nc = _build_nc()
        nc.finalize()
        _CACHE["nc"] = nc
    return _CACHE["nc"]


def make_in_maps(x, w_gate, w_up, w_down):
    xb = x.astype(BF16_NP)
    wgb = w_gate.astype(BF16_NP)
    wub = w_up.astype(BF16_NP)
    wdb = w_down.astype(BF16_NP)
    in_maps = []
    for c in range(NCORES):
        e0 = c * EPC
        # xq[e,p,a,t] = x[e*TPE+t, a*P+p]
        xs = xb[e0 * TPE:(e0 + EPC) * TPE].reshape(EPC, TPE, HT, P)
        xqc = np.ascontiguousarray(xs.transpose(0, 3, 2, 1))
        # wgq[e,f,p,a*P+j] = w_gate[e, f*P+j, a*P+p]
        wg = wgb[e0:e0 + EPC].reshape(EPC, FT, P, HT, P)
        wgc = np.ascontiguousarray(wg.transpose(0, 1, 4, 3, 2)).reshape(
            EPC, FT, P, HT * P)
        wu = wub[e0:e0 + EPC].reshape(EPC, FT, P, HT, P)
        wuc = np.ascontiguousarray(wu.transpose(0, 1, 4, 3, 2)).reshape(
            EPC, FT, P, HT * P)
        # wdq[e,g,p,f*P+j] = w_down[e, g*P+j, f*P+p]
        wd = wdb[e0:e0 + EPC].reshape(EPC, HGS, P, FT, P)
        wdc = np.ascontiguousarray(wd.transpose(0, 1, 4, 3, 2)).reshape(
            EPC, HGS, P, FT * P)
        in_maps.append({"xq": xqc, "wgq": wgc, "wuq": wuc, "wdq": wdc})
    return in_maps


def _numpy_fallback(x, w_gate, w_up, w_down, counts):
    out = np.empty((x.shape[0], w_down.shape[1]), np.float32)
    o = 0
    for e in range(len(counts)):
        n = int(counts[e])
        xi = x[o:o + n]
        gate = xi @ w_gate[e].T
        up = xi @ w_up[e].T
        hidden = (gate / (1.0 + np.exp(-gate))) * up
        out[o:o + n] = hidden @ w_down[e].T
        o += n
    return out


def kernel(x, w_gate, w_up, w_down, tokens_per_expert):
    global LAST_RESULTS
    x = np.asarray(x, dtype=np.float32)
    w_gate = np.asarray(w_gate, dtype=np.float32)
    w_up = np.asarray(w_up, dtype=np.float32)
    w_down = np.asarray(w_down, dtype=np.float32)
    counts = np.asarray(tokens_per_expert).astype(np.int64)

    if not (counts.shape == (E,) and np.all(counts == TPE)):
        # Non-uniform routing: the compiled program is shaped for the
        # uniform split the reference generator produces.
        return _numpy_fallback(x, w_gate, w_up, w_down, counts)

    nc = get_nc()
    res = run_bass_kernel_spmd(
        nc, make_in_maps(x, w_gate, w_up, w_down), list(range(NCORES)),
        trace=TRACE, **TRACE_KW,
    )
    LAST_RESULTS = res
    out = np.empty((T, H), np.float32)
    for c in range(NCORES):
        o = res.results[c]["outq"]  # [EPC, HGS, P, TPE]
        for el in range(EPC):
            t0 = (c * EPC + el) * TPE
            # out[t0+t, g*P+p] = o[el, g, p, t]
            out[t0:t0 + TPE] = o[el].transpose(2, 0, 1).reshape(TPE, H)
    return out


# revision 7
# speedup vs baseline: 1.0792x; 1.0055x over previous
"""Grouped SwiGLU MoE MLP (16 experts) on 8 NeuronCores, expert-parallel.

Reference computation, per expert e over its contiguous token slice xi:
    out = (silu(xi @ w_gate[e].T) * (xi @ w_up[e].T)) @ w_down[e].T

Sharding: expert-parallel. Core c owns experts {2c, 2c+1}; the host hands it
the matching contiguous 2048-token slice of x (tokens are pre-sorted by
expert), so no device-side collectives are needed.

All matmul operands are bf16 (host-cast): the PE streaming rate matches
fp32r but FWL halves LDWEIGHTS time and input DMA bytes halve. PSUM
accumulation stays fp32 and the output is fp32 (~4e-3 rel err, inside the
2e-2 gate).

Layout: pre-packed on the host so every DMA is a contiguous block:
  xq  [EPC, P, HT, TPE]   xq[e,p,a,t]  = x[e*TPE+t, a*P+p]
  wgq [EPC, FT, P, HT*P]  wgq[e,f,p,a*P+j] = w_gate[e, f*P+j, a*P+p]
  wuq  same as wgq for w_up
  wdq [EPC, HG, P, FT*P]  wdq[e,g,p,f*P+j] = w_down[e, g*P+j, f*P+p]
  outq[EPC, HG, P, TPE]   outq[e,g,p,t] = out[e*TPE+t, g*P+p]

Schedule (from trace analysis of earlier revisions):
- The framework preamble ends ~6.5us and the first DMA packet lands ~8us;
  NWARM dummy matmuls on a zeroed tile hold HAM at K=8/8 through that
  window so real work starts at 2.4GHz.
- Chains are t-half-outer so the first f-tile only needs the th=0 half of
  x (1MB instead of 2MB) before the PE can run 32 back-to-back matmuls,
  and so each t-half's PSUM pair drains while the other half's chains run.
- The Scalar queue runs ONLY the silu ACTIVATEs: a dma_start whose
  pool-pacing semaphore isn't yet satisfied parks its whole queue, and
  parking the silu queue delays PSUM consumption and costs the PE one
  matmul slot per group. All steady-state DMA goes on the Sync ring,
  ordered so no paced load ever sits ahead of a store it would block.
- Expert 0's first-tile weights + x are split across both rings for
  startup bandwidth (the Scalar queue is empty until the first silu).
"""

import numpy as np
import ml_dtypes

import concourse.bass as bass
import concourse.bacc as bacc
import concourse.mybir as mybir
from concourse import tile
from concourse.bass_utils import run_bass_kernel_spmd

E, T, H, F = 16, 16384, 1024, 2048
NCORES = 8
EPC = E // NCORES          # experts per core
TPE = T // E               # tokens per expert (uniform fast path)
P = 128                    # SBUF partitions
HT = H // P                # 8 h-tiles (contraction tiles for gate/up)
FT = F // P                # 16 f-tiles
HGS = H // P               # 8 output h-groups for down proj
NT = 512                   # matmul moving free dim (PSUM bank = 512 fp32)
TH = TPE // NT             # 2 t-halves
NWARM = 20                 # dummy matmuls to pre-warm the PE clock

BF16 = mybir.dt.bfloat16
F32 = mybir.dt.float32
BF16_NP = ml_dtypes.bfloat16

_CACHE = {}

# Set by run for test harness introspection (exec_time_ns, profile).
LAST_RESULTS = None
TRACE = False
TRACE_KW = {}


def _build_nc():
    nc = bacc.Bacc()
    xq = nc.dram_tensor("xq", [EPC, P, HT, TPE], BF16, kind="ExternalInput")
    wgq = nc.dram_tensor("wgq", [EPC, FT, P, HT * P], BF16, kind="ExternalInput")
    wuq = nc.dram_tensor("wuq", [EPC, FT, P, HT * P], BF16, kind="ExternalInput")
    wdq = nc.dram_tensor("wdq", [EPC, HGS, P, FT * P], BF16, kind="ExternalInput")
    outq = nc.dram_tensor("outq", [EPC, HGS, P, TPE], F32, kind="ExternalOutput")

    with tile.TileContext(nc) as tc:
        with (
            tc.tile_pool(name="xp", bufs=2) as xp,
            tc.tile_pool(name="wgp", bufs=5) as wgp,
            tc.tile_pool(name="wup", bufs=5) as wup,
            tc.tile_pool(name="wdp", bufs=8) as wdp,
            tc.tile_pool(name="hid", bufs=FT + 1) as hidp,
            tc.tile_pool(name="tmp", bufs=5) as tmpp,
            tc.tile_pool(name="osb", bufs=4) as osbp,
            tc.tile_pool(name="ps", bufs=8, space=bass.MemorySpace.PSUM) as psp,
        ):
            # PE warm-up (see module docstring). The memset runs on the DVE,
            # whose preamble finishes earliest and which is otherwise idle
            # until the first mul.
            warm = tmpp.tile([P, NT], BF16, tag="tmp", name="warm")
            nc.vector.memset(warm[:], 0.0)
            wps = psp.tile([P, NT], F32, tag="ps", name="warm_ps")
            for _ in range(NWARM):
                nc.tensor.matmul(wps[:], warm[:, 0:P], warm[:],
                                 start=True, stop=True)

            xts = {}
            wgts = {}
            wuts = {}

            def prefetch_head(el):
                """First f-tile weights + x. For expert 0 this is the
                startup-critical burst and is split across both rings; the
                Scalar queue is otherwise empty until the first silu."""
                xt = xp.tile([P, HT, TPE], BF16, tag="xt", name=f"x{el}")
                xts[el] = xt
                wgt = wgp.tile([P, HT * P], BF16, tag="wg", name=f"wg{el}_0")
                nc.sync.dma_start(wgt[:], wgq[el, 0])
                wgts[(el, 0)] = wgt
                # x's th=0 half before wu: the up-chain consumes wu only
                # 8 matmuls after the gate chain starts, but every chain
                # needs x.
                nc.scalar.dma_start(xt[:, 4:8, 0:NT], xq[el][:, 4:8, 0:NT])
                wut = wup.tile([P, HT * P], BF16, tag="wu", name=f"wu{el}_0")
                nc.scalar.dma_start(wut[:], wuq[el, 0])
                wuts[(el, 0)] = wut
                nc.sync.dma_start(xt[:, 0:4, 0:NT], xq[el][:, 0:4, 0:NT])
                nc.sync.dma_start(xt[:, 0:4, NT:TPE], xq[el][:, 0:4, NT:TPE])
                nc.scalar.dma_start(xt[:, 4:8, NT:TPE], xq[el][:, 4:8, NT:TPE])

            prefetch_head(0)
            for el in range(EPC):
                xt = xts[el]
                for ft in range(1, FT):
                    wgt = wgp.tile([P, HT * P], BF16, tag="wg", name=f"wg{el}_{ft}")
                    nc.sync.dma_start(wgt[:], wgq[el, ft])
                    wgts[(el, ft)] = wgt
                    wut = wup.tile([P, HT * P], BF16, tag="wu", name=f"wu{el}_{ft}")
                    nc.sync.dma_start(wut[:], wuq[el, ft])
                    wuts[(el, ft)] = wut
                wdts = {}
                for hg in range(HGS):
                    wdt = wdp.tile([P, FT * P], BF16, tag="wd", name=f"wd{el}_{hg}")
                    nc.sync.dma_start(wdt[:], wdq[el, hg])
                    wdts[hg] = wdt

                # Gate/up, t-half-outer: for each f-tile, run the gate and
                # up chains for th=0 (16 MMs), whose silu+mul drain while
                # the th=1 chains (16 MMs) run.
                hids = []
                for ft in range(FT):
                    wgt, wut = wgts.pop((el, ft)), wuts.pop((el, ft))
                    hid = hidp.tile([P, TPE], BF16, tag="hid", name=f"hid{el}_{ft}")
                    for th in range(TH):
                        tsl = slice(th * NT, (th + 1) * NT)
                        g_ps = psp.tile([P, NT], F32, tag="ps", name=f"g{el}_{ft}_{th}")
                        u_ps = psp.tile([P, NT], F32, tag="ps", name=f"u{el}_{ft}_{th}")
                        for ht in range(HT):
                            nc.tensor.matmul(
                                g_ps[:], wgt[:, ht * P:(ht + 1) * P],
                                xt[:, ht, tsl],
                                start=(ht == 0), stop=(ht == HT - 1),
                            )
                        for ht in range(HT):
                            nc.tensor.matmul(
                                u_ps[:], wut[:, ht * P:(ht + 1) * P],
                                xt[:, ht, tsl],
                                start=(ht == 0), stop=(ht == HT - 1),
                            )
                        tmp = tmpp.tile([P, NT], BF16, tag="tmp")
                        nc.scalar.activation(
                            tmp[:], g_ps[:],
                            mybir.ActivationFunctionType.Silu,
                        )
                        nc.vector.tensor_mul(hid[:, tsl], tmp[:], u_ps[:])
                    hids.append(hid)

                # Next expert's head prefetch goes out before this expert's
                # output stores so its x/weights are resident at the
                # expert boundary.
                if el + 1 < EPC:
                    prefetch_head(el + 1)

                # Down projection, t-half-outer: th=0's 16-MM chain
                # completes before th=1's starts, so its copy+store overlap
                # the th=1 chain and the post-loop tail is one store deep.
                for hg in range(HGS):
                    wdt = wdts[hg]
                    for th in range(TH):
                        tsl = slice(th * NT, (th + 1) * NT)
                        d_ps = psp.tile([P, NT], F32, tag="ps", name=f"d{el}_{hg}_{th}")
                        for ft in range(FT):
                            nc.tensor.matmul(
                                d_ps[:], wdt[:, ft * P:(ft + 1) * P],
                                hids[ft][:, tsl],
                                start=(ft == 0), stop=(ft == FT - 1),
                            )
                        osb = osbp.tile([P, NT], F32, tag="osb")
                        if el == EPC - 1 and hg == HGS - 1 and th == TH - 1:
                            # Final readout: halve the tail by copying on
                            # DVE+ScalarE in parallel and storing the halves
                            # on both HWDGE queues (each queue moves a lone
                            # 256KB store at only ~95GB/s).
                            nc.vector.tensor_copy(osb[:, 0:NT // 2],
                                                  d_ps[:, 0:NT // 2])
                            nc.scalar.activation(
                                osb[:, NT // 2:NT], d_ps[:, NT // 2:NT],
                                mybir.ActivationFunctionType.Copy,
                            )
                            half = slice(th * NT, th * NT + NT // 2)
                            nc.sync.dma_start(outq[el, hg][:, half],
                                              osb[:, 0:NT // 2])
                            half2 = slice(th * NT + NT // 2, (th + 1) * NT)
                            nc.scalar.dma_start(outq[el, hg][:, half2],
                                                osb[:, NT // 2:NT])
                        else:
                            nc.vector.tensor_copy(osb[:], d_ps[:])
                            nc.sync.dma_start(outq[el, hg][:, tsl], osb[:])
    return nc


def get_nc():
    if "nc" not in _CACHE:
        nc = _build_nc()
        nc.finalize()
        _CACHE["nc"] = nc
    return _CACHE["nc"]


def make_in_maps(x, w_gate, w_up, w_down):
    xb = x.astype(BF16_NP)
    wgb = w_gate.astype(BF16_NP)
    wub = w_up.astype(BF16_NP)
    wdb = w_down.astype(BF16_NP)
    in_maps = []
    for c in range(NCORES):
        e0 = c * EPC
        # xq[e,p,a,t] = x[e*TPE+t, a*P+p]
        xs = xb[e0 * TPE:(e0 + EPC) * TPE].reshape(EPC, TPE, HT, P)
        xqc = np.ascontiguousarray(xs.transpose(0, 3, 2, 1))
        # wgq[e,f,p,a*P+j] = w_gate[e, f*P+j, a*P+p]
        wg = wgb[e0:e0 + EPC].reshape(EPC, FT, P, HT, P)
        wgc = np.ascontiguousarray(wg.transpose(0, 1, 4, 3, 2)).reshape(
            EPC, FT, P, HT * P)
        wu = wub[e0:e0 + EPC].reshape(EPC, FT, P, HT, P)
        wuc = np.ascontiguousarray(wu.transpose(0, 1, 4, 3, 2)).reshape(
            EPC, FT, P, HT * P)
        # wdq[e,g,p,f*P+j] = w_down[e, g*P+j, f*P+p]
        wd = wdb[e0:e0 + EPC].reshape(EPC, HGS, P, FT, P)
        wdc = np.ascontiguousarray(wd.transpose(0, 1, 4, 3, 2)).reshape(
            EPC, HGS, P, FT * P)
        in_maps.append({"xq": xqc, "wgq": wgc, "wuq": wuc, "wdq": wdc})
    return in_maps


def _numpy_fallback(x, w_gate, w_up, w_down, counts):
    out = np.empty((x.shape[0], w_down.shape[1]), np.float32)
    o = 0
    for e in range(len(counts)):
        n = int(counts[e])
        xi = x[o:o + n]
        gate = xi @ w_gate[e].T
        up = xi @ w_up[e].T
        hidden = (gate / (1.0 + np.exp(-gate))) * up
        out[o:o + n] = hidden @ w_down[e].T
        o += n
    return out


def kernel(x, w_gate, w_up, w_down, tokens_per_expert):
    global LAST_RESULTS
    x = np.asarray(x, dtype=np.float32)
    w_gate = np.asarray(w_gate, dtype=np.float32)
    w_up = np.asarray(w_up, dtype=np.float32)
    w_down = np.asarray(w_down, dtype=np.float32)
    counts = np.asarray(tokens_per_expert).astype(np.int64)

    if not (counts.shape == (E,) and np.all(counts == TPE)):
        # Non-uniform routing: the compiled program is shaped for the
        # uniform split the reference generator produces.
        return _numpy_fallback(x, w_gate, w_up, w_down, counts)

    nc = get_nc()
    res = run_bass_kernel_spmd(
        nc, make_in_maps(x, w_gate, w_up, w_down), list(range(NCORES)),
        trace=TRACE, **TRACE_KW,
    )
    LAST_RESULTS = res
    out = np.empty((T, H), np.float32)
    for c in range(NCORES):
        o = res.results[c]["outq"]  # [EPC, HGS, P, TPE]
        for el in range(EPC):
            t0 = (c * EPC + el) * TPE
            # out[t0+t, g*P+p] = o[el, g, p, t]
            out[t0:t0 + TPE] = o[el].transpose(2, 0, 1).reshape(TPE, H)
    return out


# revision 9
# speedup vs baseline: 1.0801x; 1.0008x over previous
"""Grouped SwiGLU MoE MLP (16 experts) on 8 NeuronCores, expert-parallel.

Reference computation, per expert e over its contiguous token slice xi:
    out = (silu(xi @ w_gate[e].T) * (xi @ w_up[e].T)) @ w_down[e].T

Sharding: expert-parallel. Core c owns experts {2c, 2c+1}; the host hands it
the matching contiguous 2048-token slice of x (tokens are pre-sorted by
expert), so no device-side collectives are needed.

All matmul operands are bf16 (host-cast): the PE streaming rate matches
fp32r but FWL halves LDWEIGHTS time and input DMA bytes halve. PSUM
accumulation stays fp32 and the output is fp32 (~4e-3 rel err, inside the
2e-2 gate).

Layout: pre-packed on the host so every DMA is a contiguous block:
  xq  [EPC, P, HT, TPE]   xq[e,p,a,t]  = x[e*TPE+t, a*P+p]
  wgq [EPC, FT, P, HT*P]  wgq[e,f,p,a*P+j] = w_gate[e, f*P+j, a*P+p]
  wuq  same as wgq for w_up
  wdq [EPC, HG, P, FT*P]  wdq[e,g,p,f*P+j] = w_down[e, g*P+j, f*P+p]
  outq[EPC, HG, P, TPE]   outq[e,g,p,t] = out[e*TPE+t, g*P+p]

Schedule (from trace analysis of earlier revisions):
- The framework preamble ends ~6.5us and the first DMA packet lands ~8us;
  NWARM dummy matmuls on a zeroed tile hold HAM at K=8/8 through that
  window so real work starts at 2.4GHz.
- Chains are t-half-outer so the first f-tile only needs the th=0 half of
  x (1MB instead of 2MB) before the PE can run 32 back-to-back matmuls,
  and so each t-half's PSUM pair drains while the other half's chains run.
- The Scalar queue runs ONLY the silu ACTIVATEs: a dma_start whose
  pool-pacing semaphore isn't yet satisfied parks its whole queue, and
  parking the silu queue delays PSUM consumption and costs the PE one
  matmul slot per group. All steady-state DMA goes on the Sync ring,
  ordered so no paced load ever sits ahead of a store it would block.
- Expert 0's first-tile weights + x are split across both rings for
  startup bandwidth (the Scalar queue is empty until the first silu).
"""

import numpy as np
import ml_dtypes

import concourse.bass as bass
import concourse.bacc as bacc
import concourse.mybir as mybir
from concourse import tile
from concourse.bass_utils import run_bass_kernel_spmd

E, T, H, F = 16, 16384, 1024, 2048
NCORES = 8
EPC = E // NCORES          # experts per core
TPE = T // E               # tokens per expert (uniform fast path)
P = 128                    # SBUF partitions
HT = H // P                # 8 h-tiles (contraction tiles for gate/up)
FT = F // P                # 16 f-tiles
HGS = H // P               # 8 output h-groups for down proj
NT = 512                   # matmul moving free dim (PSUM bank = 512 fp32)
TH = TPE // NT             # 2 t-halves
NWARM = 20                 # dummy matmuls to pre-warm the PE clock

BF16 = mybir.dt.bfloat16
F32 = mybir.dt.float32
BF16_NP = ml_dtypes.bfloat16

_CACHE = {}

# Set by run for test harness introspection (exec_time_ns, profile).
LAST_RESULTS = None
TRACE = False
TRACE_KW = {}


def _build_nc():
    nc = bacc.Bacc()
    xq = nc.dram_tensor("xq", [EPC, P, HT, TPE], BF16, kind="ExternalInput")
    wgq = nc.dram_tensor("wgq", [EPC, FT, P, HT * P], BF16, kind="ExternalInput")
    wuq = nc.dram_tensor("wuq", [EPC, FT, P, HT * P], BF16, kind="ExternalInput")
    wdq = nc.dram_tensor("wdq", [EPC, HGS, P, FT * P], BF16, kind="ExternalInput")
    outq = nc.dram_tensor("outq", [EPC, HGS, P, TPE], F32, kind="ExternalOutput")

    with tile.TileContext(nc) as tc:
        with (
            tc.tile_pool(name="xp", bufs=2) as xp,
            tc.tile_pool(name="wgp", bufs=5) as wgp,
            tc.tile_pool(name="wup", bufs=5) as wup,
            tc.tile_pool(name="wdp", bufs=8) as wdp,
            tc.tile_pool(name="hid", bufs=FT + 1) as hidp,
            tc.tile_pool(name="tmp", bufs=5) as tmpp,
            tc.tile_pool(name="osb", bufs=4) as osbp,
            tc.tile_pool(name="ps", bufs=8, space=bass.MemorySpace.PSUM) as psp,
        ):
            # PE warm-up (see module docstring). The memset runs on the DVE,
            # whose preamble finishes earliest and which is otherwise idle
            # until the first mul.
            warm = tmpp.tile([P, NT], BF16, tag="tmp", name="warm")
            nc.vector.memset(warm[:], 0.0)
            wps = psp.tile([P, NT], F32, tag="ps", name="warm_ps")
            for _ in range(NWARM):
                nc.tensor.matmul(wps[:], warm[:, 0:P], warm[:],
                                 start=True, stop=True)

            xts = {}
            wgts = {}
            wuts = {}

            def prefetch_head(el):
                """First f-tile weights + x. For expert 0 this is the
                startup-critical burst and is split across both rings; the
                Scalar queue is otherwise empty until the first silu."""
                xt = xp.tile([P, HT, TPE], BF16, tag="xt", name=f"x{el}")
                xts[el] = xt
                wgt = wgp.tile([P, HT * P], BF16, tag="wg", name=f"wg{el}_0")
                nc.sync.dma_start(wgt[:], wgq[el, 0])
                wgts[(el, 0)] = wgt
                # x's th=0 half before wu: the up-chain consumes wu only
                # 8 matmuls after the gate chain starts, but every chain
                # needs x.
                nc.scalar.dma_start(xt[:, 4:8, 0:NT], xq[el][:, 4:8, 0:NT])
                wut = wup.tile([P, HT * P], BF16, tag="wu", name=f"wu{el}_0")
                nc.scalar.dma_start(wut[:], wuq[el, 0])
                wuts[(el, 0)] = wut
                nc.sync.dma_start(xt[:, 0:4, 0:NT], xq[el][:, 0:4, 0:NT])
                nc.sync.dma_start(xt[:, 0:4, NT:TPE], xq[el][:, 0:4, NT:TPE])
                nc.scalar.dma_start(xt[:, 4:8, NT:TPE], xq[el][:, 4:8, NT:TPE])

            prefetch_head(0)
            for el in range(EPC):
                xt = xts[el]
                for ft in range(1, FT):
                    wgt = wgp.tile([P, HT * P], BF16, tag="wg", name=f"wg{el}_{ft}")
                    nc.sync.dma_start(wgt[:], wgq[el, ft])
                    wgts[(el, ft)] = wgt
                    wut = wup.tile([P, HT * P], BF16, tag="wu", name=f"wu{el}_{ft}")
                    nc.sync.dma_start(wut[:], wuq[el, ft])
                    wuts[(el, ft)] = wut
                wdts = {}
                for hg in range(HGS):
                    wdt = wdp.tile([P, FT * P], BF16, tag="wd", name=f"wd{el}_{hg}")
                    nc.sync.dma_start(wdt[:], wdq[el, hg])
                    wdts[hg] = wdt

                # Gate/up, t-half-outer: for each f-tile, run the gate and
                # up chains for th=0 (16 MMs), whose silu+mul drain while
                # the th=1 chains (16 MMs) run.
                hids = []
                for ft in range(FT):
                    wgt, wut = wgts.pop((el, ft)), wuts.pop((el, ft))
                    hid = hidp.tile([P, TPE], BF16, tag="hid", name=f"hid{el}_{ft}")
                    for th in range(TH):
                        tsl = slice(th * NT, (th + 1) * NT)
                        g_ps = psp.tile([P, NT], F32, tag="ps", name=f"g{el}_{ft}_{th}")
                        u_ps = psp.tile([P, NT], F32, tag="ps", name=f"u{el}_{ft}_{th}")
                        for ht in range(HT):
                            nc.tensor.matmul(
                                g_ps[:], wgt[:, ht * P:(ht + 1) * P],
                                xt[:, ht, tsl],
                                start=(ht == 0), stop=(ht == HT - 1),
                            )
                        for ht in range(HT):
                            nc.tensor.matmul(
                                u_ps[:], wut[:, ht * P:(ht + 1) * P],
                                xt[:, ht, tsl],
                                start=(ht == 0), stop=(ht == HT - 1),
                            )
                        tmp = tmpp.tile([P, NT], BF16, tag="tmp")
                        nc.scalar.activation(
                            tmp[:], g_ps[:],
                            mybir.ActivationFunctionType.Silu,
                        )
                        nc.vector.tensor_mul(hid[:, tsl], tmp[:], u_ps[:])
                    hids.append(hid)

                # Next expert's head prefetch goes out before this expert's
                # output stores so its x/weights are resident at the
                # expert boundary.
                if el + 1 < EPC:
                    prefetch_head(el + 1)

                # Down projection, t-half-outer: th=0's 16-MM chain
                # completes before th=1's starts, so its copy+store overlap
                # the th=1 chain and the post-loop tail is one store deep.
                for hg in range(HGS):
                    wdt = wdts[hg]
                    for th in range(TH):
                        tsl = slice(th * NT, (th + 1) * NT)
                        if el == EPC - 1 and hg == HGS - 1 and th == TH - 1:
                            # Final t-half: two N=256 chains into SEPARATE
                            # PSUM banks (a shared bank would make the first
                            # chain's copy collide with the second chain's
                            # writes), so the first quarter's store overlaps
                            # the second chain and the post-loop tail is one
                            # 64KB-store deep per queue.
                            NQ = NT // 2
                            d_qs = [psp.tile([P, NT], F32, tag="ps",
                                             name=f"d{el}_{hg}_{th}_{q}")
                                    for q in range(2)]
                            osb = osbp.tile([P, NT], F32, tag="osb")
                            for q in range(2):
                                qsl = slice(q * NQ, (q + 1) * NQ)
                                for ft in range(FT):
                                    nc.tensor.matmul(
                                        d_qs[q][:, 0:NQ],
                                        wdt[:, ft * P:(ft + 1) * P],
                                        hids[ft][:, th * NT + q * NQ:
                                                  th * NT + (q + 1) * NQ],
                                        start=(ft == 0), stop=(ft == FT - 1),
                                    )
                                if q == 0:
                                    nc.vector.tensor_copy(osb[:, qsl],
                                                          d_qs[0][:, 0:NQ])
                                    nc.scalar.dma_start(
                                        outq[el, hg][:, th * NT:th * NT + NQ],
                                        osb[:, qsl])
                            nc.vector.tensor_copy(osb[:, NQ:NQ + NQ // 2],
                                                  d_qs[1][:, 0:NQ // 2])
                            nc.scalar.activation(
                                osb[:, NQ + NQ // 2:NT],
                                d_qs[1][:, NQ // 2:NQ],
                                mybir.ActivationFunctionType.Copy,
                            )
                            nc.sync.dma_start(
                                outq[el, hg][:, th * NT + NQ:
                                             th * NT + NQ + NQ // 2],
                                osb[:, NQ:NQ + NQ // 2])
                            nc.scalar.dma_start(
                                outq[el, hg][:, th * NT + NQ + NQ // 2:
                                             (th + 1) * NT],
                                osb[:, NQ + NQ // 2:NT])
                        else:
                            d_ps = psp.tile([P, NT], F32, tag="ps",
                                            name=f"d{el}_{hg}_{th}")
                            for ft in range(FT):
                                nc.tensor.matmul(
                                    d_ps[:], wdt[:, ft * P:(ft + 1) * P],
                                    hids[ft][:, tsl],
                                    start=(ft == 0), stop=(ft == FT - 1),
                                )
                            osb = osbp.tile([P, NT], F32, tag="osb")
                            nc.vector.tensor_copy(osb[:], d_ps[:])
                            nc.sync.dma_start(outq[el, hg][:, tsl], osb[:])
    return nc


def get_nc():
    if "nc" not in _CACHE:
        nc = _build_nc()
        nc.finalize()
        _CACHE["nc"] = nc
    return _CACHE["nc"]


def make_in_maps(x, w_gate, w_up, w_down):
    xb = x.astype(BF16_NP)
    wgb = w_gate.astype(BF16_NP)
    wub = w_up.astype(BF16_NP)
    wdb = w_down.astype(BF16_NP)
    in_maps = []
    for c in range(NCORES):
        e0 = c * EPC
        # xq[e,p,a,t] = x[e*TPE+t, a*P+p]
        xs = xb[e0 * TPE:(e0 + EPC) * TPE].reshape(EPC, TPE, HT, P)
        xqc = np.ascontiguousarray(xs.transpose(0, 3, 2, 1))
        # wgq[e,f,p,a*P+j] = w_gate[e, f*P+j, a*P+p]
        wg = wgb[e0:e0 + EPC].reshape(EPC, FT, P, HT, P)
        wgc = np.ascontiguousarray(wg.transpose(0, 1, 4, 3, 2)).reshape(
            EPC, FT, P, HT * P)
        wu = wub[e0:e0 + EPC].reshape(EPC, FT, P, HT, P)
        wuc = np.ascontiguousarray(wu.transpose(0, 1, 4, 3, 2)).reshape(
            EPC, FT, P, HT * P)
        # wdq[e,g,p,f*P+j] = w_down[e, g*P+j, f*P+p]
        wd = wdb[e0:e0 + EPC].reshape(EPC, HGS, P, FT, P)
        wdc = np.ascontiguousarray(wd.transpose(0, 1, 4, 3, 2)).reshape(
            EPC, HGS, P, FT * P)
        in_maps.append({"xq": xqc, "wgq": wgc, "wuq": wuc, "wdq": wdc})
    return in_maps


def _numpy_fallback(x, w_gate, w_up, w_down, counts):
    out = np.empty((x.shape[0], w_down.shape[1]), np.float32)
    o = 0
    for e in range(len(counts)):
        n = int(counts[e])
        xi = x[o:o + n]
        gate = xi @ w_gate[e].T
        up = xi @ w_up[e].T
        hidden = (gate / (1.0 + np.exp(-gate))) * up
        out[o:o + n] = hidden @ w_down[e].T
        o += n
    return out


def kernel(x, w_gate, w_up, w_down, tokens_per_expert):
    global LAST_RESULTS
    x = np.asarray(x, dtype=np.float32)
    w_gate = np.asarray(w_gate, dtype=np.float32)
    w_up = np.asarray(w_up, dtype=np.float32)
    w_down = np.asarray(w_down, dtype=np.float32)
    counts = np.asarray(tokens_per_expert).astype(np.int64)

    if not (counts.shape == (E,) and np.all(counts == TPE)):
        # Non-uniform routing: the compiled program is shaped for the
        # uniform split the reference generator produces.
        return _numpy_fallback(x, w_gate, w_up, w_down, counts)

    nc = get_nc()
    res = run_bass_kernel_spmd(
        nc, make_in_maps(x, w_gate, w_up, w_down), list(range(NCORES)),
        trace=TRACE, **TRACE_KW,
    )
    LAST_RESULTS = res
    out = np.empty((T, H), np.float32)
    for c in range(NCORES):
        o = res.results[c]["outq"]  # [EPC, HGS, P, TPE]
        for el in range(EPC):
            t0 = (c * EPC + el) * TPE
            # out[t0+t, g*P+p] = o[el, g, p, t]
            out[t0:t0 + TPE] = o[el].transpose(2, 0, 1).reshape(TPE, H)
    return out
